# revision 7
# baseline (speedup 1.0000x reference)
"""Trainium2 Bass kernel for a CAM (channel-attention) module — fp8 edition.

Computes, per batch b:
    E = X @ X^T                      (C x C channel energy, X = x[b] in R^{C x L})
    A = softmax(rowmax(E) - E)       (== softmax(-E) row-wise, stabilized)
    y[b] = gamma * (A @ X) + x[b]

Shapes: x [32, 512, 4096] f32, gamma [1] f32.  Data-parallel over batch:
8 NeuronCores x 4 batches each.  No cross-core communication.

v2 changes vs baseline:
  - All HBM I/O narrowed: x residual/bf16 in, xt + x in fp8e4 for the two
    matmuls, y out in bf16 (upcast to f32 on host).  50.4 MB/core vs 84.
  - Both matmuls run fp8 DoubleRow (2 contraction rows per PE cell, 2x
    MAC throughput): mm1 contracts l-tile pairs of the host-pretiled
    xt8 [128, NLT, C]; mm2 contracts d-chunk pairs with PT pair tiles
    [128, 2, C] built from PE transposes of the fp8 softmax output.
  - y stores issued from GpSimd (SWDGE) so the SP/ACT queues keep
    feeding input DMAs / activations.

v6: xt loads are split into halves (mm1 starts at half-arrival) and
prefetched one batch ahead, closing the inter-batch PE gaps seen in the
NTFF profile.
"""

import numpy as np
import ml_dtypes

B, C, L = 32, 512, 4096
N_CORES = 8
BPC = B // N_CORES  # batches per core

_CACHE: dict = {}

IN_NAMES = ["xb", "xt8", "x8", "gamma"]
OUT_NAME = "y"


def build_nc(bpc: int = BPC, repeat: int = 1, hw_loop: int = 0):
    from contextlib import ExitStack

    import concourse.bass as bass  # noqa: F401  (registers engines)
    import concourse.tile as tile
    from concourse import bacc, masks, mybir

    f32 = mybir.dt.float32
    bf16 = mybir.dt.bfloat16
    fp8 = mybir.dt.float8e4
    AX = mybir.AxisListType
    OP = mybir.AluOpType
    ACT = mybir.ActivationFunctionType
    DR = mybir.MatmulPerfMode.DoubleRow

    NCC = C // 128  # 4 c-chunks (partition blocks of C)
    NLT = L // 128  # 32 l-tiles (contraction tiles for mm1)
    NLP = NLT // 2  # 16 l-tile pairs (DoubleRow contraction steps)
    NDP = NCC // 2  # 2 d-chunk pairs for mm2

    nc = bacc.Bacc("TRN2", target_bir_lowering=False, debug=False, num_devices=N_CORES)
    xbd = nc.dram_tensor("xb", [bpc, C, L], bf16, kind="ExternalInput")
    xtd = nc.dram_tensor("xt8", [bpc, 128, NLT * C], fp8, kind="ExternalInput")
    x8d = nc.dram_tensor("x8", [bpc, C, L], fp8, kind="ExternalInput")
    gd = nc.dram_tensor("gamma", [1, 1], f32, kind="ExternalInput")
    yd = nc.dram_tensor("y", [bpc, C, L], bf16, kind="ExternalOutput")

    with tile.TileContext(nc) as tc, ExitStack() as ctx:
        const = ctx.enter_context(tc.tile_pool(name="const", bufs=1))
        xt_pool = ctx.enter_context(tc.tile_pool(name="xt", bufs=4))
        xb_pool = ctx.enter_context(tc.tile_pool(name="xb", bufs=4))
        x8_pool = ctx.enter_context(tc.tile_pool(name="x8", bufs=4))
        prow_pool = ctx.enter_context(tc.tile_pool(name="prow", bufs=9))
        pt_pool = ctx.enter_context(tc.tile_pool(name="pt", bufs=4))
        eblk_pool = ctx.enter_context(tc.tile_pool(name="eblk", bufs=12))
        out_pool = ctx.enter_context(tc.tile_pool(name="out", bufs=3))
        st_pool = ctx.enter_context(tc.tile_pool(name="stats", bufs=24))
        e_psum = ctx.enter_context(tc.tile_pool(name="e_ps", bufs=2, space="PSUM"))
        t_psum = ctx.enter_context(tc.tile_pool(name="t_ps", bufs=2, space="PSUM"))
        u_psum = ctx.enter_context(tc.tile_pool(name="u_ps", bufs=4, space="PSUM"))

        identity = const.tile([128, 128], bf16)
        masks.make_identity(nc, identity[:])
        identity_f = const.tile([128, 128], f32)
        masks.make_identity(nc, identity_f[:])
        g_sb = const.tile([1, 1], f32)
        nc.sync.dma_start(g_sb[:], gd.ap())
        gamma_bc = const.tile([128, 1], f32)
        nc.gpsimd.partition_broadcast(gamma_bc[:], g_sb[:])

        loop_cm = tc.For_i(0, hw_loop, 1) if hw_loop else None
        if loop_cm is not None:
            ctx.enter_context(loop_cm)

        HLT = NLT // 2  # l-tiles per xt half-load
        xt_tiles: dict = {}

        def load_xt(bb):
            """Half-split load of batch bb's xt tile (prefetch helper)."""
            if bb in xt_tiles or bb >= bpc:
                return
            halves = []
            for h in range(2):
                t = xt_pool.tile([128, HLT, C], fp8, name="xt_h", tag="xt_h")
                nc.sync.dma_start(
                    t[:],
                    xtd.ap()[bb, :, h * HLT * C : (h + 1) * HLT * C].rearrange(
                        "p (n c) -> p n c", c=C
                    ),
                )
                halves.append(t)
            xt_tiles[bb] = halves

        for b_rep in range(bpc * repeat):
            b = b_rep % bpc
            # --- xt load (fp8, host-pretiled [128, NLT, C]); also prefetch
            # the next batch so mm1 never waits on it ---
            if b == 0:
                xt_tiles.clear()
            load_xt(b)
            load_xt(b + 1)
            xt_lo, xt_hi = xt_tiles[b]

            # --- mm1 (upper-triangle block-columns only; E is symmetric) ---
            # DoubleRow: each step contracts an l-tile pair (256 rows).
            psc_sb = []
            t_ts = []
            eblk_sb = {}  # (dc, m) -> SBUF copy of E[dc][:, m-block]
            for m in range(NCC):
                e_t = e_psum.tile([128, C], f32)
                mm0 = None
                for i in range(NLP):
                    src = xt_lo if i < NLP // 2 else xt_hi
                    ii = i if i < NLP // 2 else i - NLP // 2
                    mm = nc.tensor.matmul(
                        e_t[:, m * 128 :],
                        lhsT=src[:, 2 * ii : 2 * ii + 2, m * 128 : (m + 1) * 128],
                        rhs=src[:, 2 * ii : 2 * ii + 2, m * 128 :],
                        start=(i == 0),
                        stop=(i == NLP - 1),
                        perf_mode=DR,
                    )
                    if i == 0:
                        mm0 = mm
                # fill columns [0:m*128] by transposing earlier chunks' blocks
                # (E is symmetric).  start=False so the per-bank has_written
                # clear of the accumulation group is not re-triggered; the
                # explicit dep keeps each transpose after that group's first
                # matmul (whose start=True clear would otherwise mark the
                # transposed columns pending-zero afterwards).
                for dc in range(m):
                    tr = nc.tensor.matmul(
                        e_t[:, dc * 128 : (dc + 1) * 128],
                        lhsT=eblk_sb.pop((dc, m))[:],
                        rhs=identity_f[:],
                        is_transpose=True,
                        start=False,
                        stop=True,
                        skip_group_check=True,
                    )
                    tile.add_dep_helper(
                        tr.ins, mm0.ins, reason="transpose after bank clear"
                    )
                # stage upper blocks needed by later chunks before e_t is freed
                for mc in range(m + 1, NCC):
                    blk = eblk_pool.tile([128, 128], f32, name="eblk", tag="eblk")
                    nc.scalar.copy(blk[:], e_t[:, mc * 128 : (mc + 1) * 128])
                    eblk_sb[(m, mc)] = blk
                m_t = st_pool.tile([128, 1], f32)
                nc.vector.tensor_reduce(m_t[:], e_t[:], axis=AX.X, op=OP.min)
                p_t = prow_pool.tile([128, C], bf16, name="p_t", tag="p_t")
                s_t = st_pool.tile([128, 1], f32)
                nc.scalar.activation(
                    p_t[:], e_t[:], ACT.Exp, bias=m_t[:], scale=-1.0, accum_out=s_t[:]
                )
                r_t = st_pool.tile([128, 1], f32)
                nc.vector.reciprocal(r_t[:], s_t[:])
                t_t = st_pool.tile([128, 1], f32, name="t_t", tag="t_t", bufs=16)
                nc.vector.tensor_scalar_mul(t_t[:], r_t[:], gamma_bc[:])
                t_ts.append(t_t)
                psc_sb.append(p_t)

            # --- x loads; emitted after mm1 so xt loads win early DMA
            # contention; consumers are mm2 (x8) and the epilogue (xb) ---
            xbp = []
            x8p = []
            for j in range(NDP):
                tb = xb_pool.tile([128, 2, L], bf16, name="xbp", tag="xbp")
                nc.sync.dma_start(
                    tb[:],
                    xbd.ap()[b, 256 * j : 256 * (j + 1), :].rearrange(
                        "(k p) l -> p k l", p=128
                    ),
                )
                xbp.append(tb)
                t8 = x8_pool.tile([128, 2, L], fp8, name="x8p", tag="x8p")
                nc.scalar.dma_start(
                    t8[:],
                    x8d.ap()[b, 256 * j : 256 * (j + 1), :].rearrange(
                        "(k p) l -> p k l", p=128
                    ),
                )
                x8p.append(t8)

            # --- transpose P -> PT pair tiles [128 d, ko, C c-cols] (fp8) ---
            pt_sb = [
                pt_pool.tile([128, 2, C], fp8, name="pt_sb", tag="pt_sb")
                for _ in range(NDP)
            ]
            for m in range(NCC):
                for i in range(NCC):
                    tp = t_psum.tile([128, 128], bf16)
                    nc.tensor.transpose(
                        tp[:], psc_sb[m][:, i * 128 : (i + 1) * 128], identity[:]
                    )
                    nc.scalar.copy(
                        pt_sb[i // 2][:, i % 2, m * 128 : (m + 1) * 128], tp[:]
                    )

            # --- mm2 (DoubleRow over d-chunk pairs) + epilogue ---
            for m in range(NCC):
                o_t = out_pool.tile([128, L], bf16, name="o_t", tag="o_t")
                for h in range(2):
                    u_ts = [
                        u_psum.tile([128, 512], f32, name="u_t", tag="u_t")
                        for _ in range(4)
                    ]
                    for j in range(NDP):
                        for q in range(4):
                            jj = h * 4 + q
                            nc.tensor.matmul(
                                u_ts[q][:],
                                lhsT=pt_sb[j][:, :, m * 128 : (m + 1) * 128],
                                rhs=x8p[j][:, :, jj * 512 : (jj + 1) * 512],
                                start=(j == 0),
                                stop=(j == NDP - 1),
                                perf_mode=DR,
                            )
                    for q in range(4):
                        jj = h * 4 + q
                        nc.vector.scalar_tensor_tensor(
                            o_t[:, jj * 512 : (jj + 1) * 512],
                            u_ts[q][:],
                            t_ts[m][:],
                            xbp[m // 2][:, m % 2, jj * 512 : (jj + 1) * 512],
                            op0=OP.mult,
                            op1=OP.add,
                        )
                nc.gpsimd.dma_start(yd.ap()[b, m * 128 : (m + 1) * 128, :], o_t[:])

    nc.compile()
    return nc


def _get_nc():
    if "nc" not in _CACHE:
        _CACHE["nc"] = build_nc(BPC)
    return _CACHE["nc"]


def _prep_inputs(x: np.ndarray, gamma: np.ndarray):
    x = np.ascontiguousarray(np.asarray(x, dtype=np.float32))
    gamma = np.asarray(gamma, dtype=np.float32).reshape(1, 1)
    NLT = L // 128
    xb = x.astype(ml_dtypes.bfloat16)
    x8 = x.astype(ml_dtypes.float8_e4m3)
    # xt8[b, p, i*C + c] = x[b, c, 128*i + p]
    xt8 = np.ascontiguousarray(
        x.reshape(B, C, NLT, 128).transpose(0, 3, 2, 1)
    ).astype(ml_dtypes.float8_e4m3).reshape(B, 128, NLT * C)
    in_maps = []
    for c in range(N_CORES):
        sl = slice(c * BPC, (c + 1) * BPC)
        in_maps.append(
            {
                "xb": np.ascontiguousarray(xb[sl]),
                "xt8": np.ascontiguousarray(xt8[sl]),
                "x8": np.ascontiguousarray(x8[sl]),
                "gamma": gamma,
            }
        )
    return in_maps


def kernel(x: np.ndarray, gamma: np.ndarray) -> np.ndarray:
    from concourse.bass_utils import run_bass_kernel_spmd

    nc = _get_nc()
    in_maps = _prep_inputs(x, gamma)
    res = run_bass_kernel_spmd(nc, in_maps, core_ids=list(range(N_CORES)))
    y = np.concatenate([res.results[c]["y"] for c in range(N_CORES)], axis=0)
    return y.astype(np.float32)


def _make_exec_jit(nc, in_specs_names, out_shape, out_dtype=ml_dtypes.bfloat16):
    """One-bass_exec jit over 8 cores, mirroring run_bass_via_pjrt."""
    import jax
    from jax.sharding import Mesh, PartitionSpec
    from jax.experimental.shard_map import shard_map
    from concourse.bass2jax import (
        _bass_exec_p,
        install_neuronx_cc_hook,
        partition_id_tensor,
    )

    install_neuronx_cc_hook()
    out_aval = jax.core.ShapedArray(out_shape, out_dtype)
    out_name = in_specs_names[-1]

    def body(*args):
        outs = _bass_exec_p.bind(
            *args,
            partition_id_tensor(),
            out_avals=(out_aval,),
            in_names=tuple(in_specs_names) + ("partition_id",),
            out_names=(out_name,),
            lowering_input_output_aliases=(),
            sim_require_finite=True,
            sim_require_nnan=True,
            nc=nc,
        )
        return outs[0]

    mesh = Mesh(np.asarray(jax.devices()[:N_CORES]), ("core",))
    spec = PartitionSpec("core")
    jitted = jax.jit(
        shard_map(
            body,
            mesh=mesh,
            in_specs=(spec,) * len(in_specs_names),
            out_specs=spec,
            check_rep=False,
        ),
        keep_unused=True,
    )
    sharding = jax.sharding.NamedSharding(mesh, spec)
    return jitted, sharding


if __name__ == "__main__":
    rng = np.random.default_rng(0)
    x = rng.standard_normal((B, C, L), dtype=np.float32)
    gamma = np.zeros((1,), np.float32)
    y = kernel(x, gamma)
    exp = x.astype(ml_dtypes.bfloat16).astype(np.float32)
    print("gamma=0 matches bf16(x):", np.array_equal(y, exp))
    print("max abs err vs x:", np.abs(y - x).max())


# revision 10
# speedup vs baseline: 1.0273x; 1.0273x over previous
"""Trainium2 Bass kernel for a CAM (channel-attention) module — fp8 edition.

Computes, per batch b:
    E = X @ X^T                      (C x C channel energy, X = x[b] in R^{C x L})
    A = softmax(rowmax(E) - E)       (== softmax(-E) row-wise, stabilized)
    y[b] = gamma * (A @ X) + x[b]

Shapes: x [32, 512, 4096] f32, gamma [1] f32.  Data-parallel over batch:
8 NeuronCores x 4 batches each.  No cross-core communication.

v2 changes vs baseline:
  - All HBM I/O narrowed: x residual/bf16 in, xt + x in fp8e4 for the two
    matmuls, y out in bf16 (upcast to f32 on host).  50.4 MB/core vs 84.
  - Both matmuls run fp8 DoubleRow (2 contraction rows per PE cell, 2x
    MAC throughput): mm1 contracts l-tile pairs of the host-pretiled
    xt8 [128, NLT, C]; mm2 contracts d-chunk pairs with PT pair tiles
    [128, 2, C] built from PE transposes of the fp8 softmax output.
  - y stores issued from GpSimd (SWDGE) so the SP/ACT queues keep
    feeding input DMAs / activations.

v6: xt loads are split into halves (mm1 starts at half-arrival) and
prefetched one batch ahead, closing the inter-batch PE gaps seen in the
NTFF profile.
"""

import numpy as np
import ml_dtypes

B, C, L = 32, 512, 4096
N_CORES = 8
BPC = B // N_CORES  # batches per core

_CACHE: dict = {}

IN_NAMES = ["xb", "xt8", "x8", "gamma"]
OUT_NAME = "y"


def build_nc(bpc: int = BPC, repeat: int = 1, hw_loop: int = 0):
    from contextlib import ExitStack

    import concourse.bass as bass  # noqa: F401  (registers engines)
    import concourse.tile as tile
    from concourse import bacc, masks, mybir

    f32 = mybir.dt.float32
    bf16 = mybir.dt.bfloat16
    fp8 = mybir.dt.float8e4
    AX = mybir.AxisListType
    OP = mybir.AluOpType
    ACT = mybir.ActivationFunctionType
    DR = mybir.MatmulPerfMode.DoubleRow

    NCC = C // 128  # 4 c-chunks (partition blocks of C)
    NLT = L // 128  # 32 l-tiles (contraction tiles for mm1)
    NLP = NLT // 2  # 16 l-tile pairs (DoubleRow contraction steps)
    NDP = NCC // 2  # 2 d-chunk pairs for mm2

    nc = bacc.Bacc("TRN2", target_bir_lowering=False, debug=False, num_devices=N_CORES)
    xbd = nc.dram_tensor("xb", [bpc, C, L], bf16, kind="ExternalInput")
    xtd = nc.dram_tensor("xt8", [bpc, 128, NLT * C], fp8, kind="ExternalInput")
    x8d = nc.dram_tensor("x8", [bpc, C, L], fp8, kind="ExternalInput")
    gd = nc.dram_tensor("gamma", [1, 1], f32, kind="ExternalInput")
    yd = nc.dram_tensor("y", [bpc, C, L], bf16, kind="ExternalOutput")

    with tile.TileContext(nc) as tc, ExitStack() as ctx:
        const = ctx.enter_context(tc.tile_pool(name="const", bufs=1))
        xt_pool = ctx.enter_context(tc.tile_pool(name="xt", bufs=4))
        xb_pool = ctx.enter_context(tc.tile_pool(name="xb", bufs=4))
        x8_pool = ctx.enter_context(tc.tile_pool(name="x8", bufs=4))
        prow_pool = ctx.enter_context(tc.tile_pool(name="prow", bufs=9))
        pt_pool = ctx.enter_context(tc.tile_pool(name="pt", bufs=4))
        eblk_pool = ctx.enter_context(tc.tile_pool(name="eblk", bufs=12))
        out_pool = ctx.enter_context(tc.tile_pool(name="out", bufs=3))
        st_pool = ctx.enter_context(tc.tile_pool(name="stats", bufs=24))
        e_psum = ctx.enter_context(tc.tile_pool(name="e_ps", bufs=2, space="PSUM"))
        t_psum = ctx.enter_context(tc.tile_pool(name="t_ps", bufs=2, space="PSUM"))
        u_psum = ctx.enter_context(tc.tile_pool(name="u_ps", bufs=4, space="PSUM"))

        identity = const.tile([128, 128], bf16)
        masks.make_identity(nc, identity[:])
        identity_f = const.tile([128, 128], f32)
        masks.make_identity(nc, identity_f[:])
        g_sb = const.tile([1, 1], f32)
        nc.sync.dma_start(g_sb[:], gd.ap())
        gamma_bc = const.tile([128, 1], f32)
        nc.gpsimd.partition_broadcast(gamma_bc[:], g_sb[:])

        loop_cm = tc.For_i(0, hw_loop, 1) if hw_loop else None
        if loop_cm is not None:
            ctx.enter_context(loop_cm)

        HLT = NLT // 2  # l-tiles per xt half-load
        xt_tiles: dict = {}

        def load_xt(bb):
            """Half-split load of batch bb's xt tile (prefetch helper)."""
            if bb in xt_tiles or bb >= bpc:
                return
            halves = []
            for h in range(2):
                t = xt_pool.tile([128, HLT, C], fp8, name="xt_h", tag="xt_h")
                nc.sync.dma_start(
                    t[:],
                    xtd.ap()[bb, :, h * HLT * C : (h + 1) * HLT * C].rearrange(
                        "p (n c) -> p n c", c=C
                    ),
                )
                halves.append(t)
            xt_tiles[bb] = halves

        pending_stage = None

        def emit_tail(bb, psc, tts, xbp_l, x8p_l):
            """PT transposes + mm2 + epilogue + y store for batch bb."""
            # --- transpose P -> PT pair tiles [128 d, ko, C c-cols] (fp8) ---
            pt_sb = [
                pt_pool.tile([128, 2, C], fp8, name="pt_sb", tag="pt_sb")
                for _ in range(NDP)
            ]
            for m in range(NCC):
                for i in range(NCC):
                    tp = t_psum.tile([128, 128], bf16, name="tp", tag="tp")
                    nc.tensor.transpose(
                        tp[:], psc[m][:, i * 128 : (i + 1) * 128], identity[:]
                    )
                    nc.scalar.copy(
                        pt_sb[i // 2][:, i % 2, m * 128 : (m + 1) * 128], tp[:]
                    )

            # --- mm2 (DoubleRow over d-chunk pairs) + epilogue ---
            for m in range(NCC):
                o_t = out_pool.tile([128, L], bf16, name="o_t", tag="o_t")
                for h in range(2):
                    u_ts = [
                        u_psum.tile([128, 512], f32, name="u_t", tag="u_t")
                        for _ in range(4)
                    ]
                    for j in range(NDP):
                        for q in range(4):
                            jj = h * 4 + q
                            nc.tensor.matmul(
                                u_ts[q][:],
                                lhsT=pt_sb[j][:, :, m * 128 : (m + 1) * 128],
                                rhs=x8p_l[j][:, :, jj * 512 : (jj + 1) * 512],
                                start=(j == 0),
                                stop=(j == NDP - 1),
                                perf_mode=DR,
                            )
                    for q in range(4):
                        jj = h * 4 + q
                        nc.vector.scalar_tensor_tensor(
                            o_t[:, jj * 512 : (jj + 1) * 512],
                            u_ts[q][:],
                            tts[m][:],
                            xbp_l[m // 2][:, m % 2, jj * 512 : (jj + 1) * 512],
                            op0=OP.mult,
                            op1=OP.add,
                        )
                nc.gpsimd.dma_start(yd.ap()[bb, m * 128 : (m + 1) * 128, :], o_t[:])

        for b_rep in range(bpc * repeat):
            b = b_rep % bpc
            # --- xt load (fp8, host-pretiled [128, NLT, C]); also prefetch
            # the next batch so mm1 never waits on it ---
            if b == 0:
                xt_tiles.clear()
            load_xt(b)
            load_xt(b + 1)
            xt_lo, xt_hi = xt_tiles[b]

            # --- mm1 (upper-triangle block-columns only; E is symmetric) ---
            # DoubleRow: each step contracts an l-tile pair (256 rows).
            psc_sb = []
            t_ts = []
            eblk_sb = {}  # (dc, m) -> SBUF copy of E[dc][:, m-block]
            for m in range(NCC):
                e_t = e_psum.tile([128, C], f32)
                mm0 = None
                for i in range(NLP):
                    src = xt_lo if i < NLP // 2 else xt_hi
                    ii = i if i < NLP // 2 else i - NLP // 2
                    mm = nc.tensor.matmul(
                        e_t[:, m * 128 :],
                        lhsT=src[:, 2 * ii : 2 * ii + 2, m * 128 : (m + 1) * 128],
                        rhs=src[:, 2 * ii : 2 * ii + 2, m * 128 :],
                        start=(i == 0),
                        stop=(i == NLP - 1),
                        perf_mode=DR,
                    )
                    if i == 0:
                        mm0 = mm
                # fill columns [0:m*128] by transposing earlier chunks' blocks
                # (E is symmetric).  start=False so the per-bank has_written
                # clear of the accumulation group is not re-triggered; the
                # explicit dep keeps each transpose after that group's first
                # matmul (whose start=True clear would otherwise mark the
                # transposed columns pending-zero afterwards).
                for dc in range(m):
                    tr = nc.tensor.matmul(
                        e_t[:, dc * 128 : (dc + 1) * 128],
                        lhsT=eblk_sb.pop((dc, m))[:],
                        rhs=identity_f[:],
                        is_transpose=True,
                        start=False,
                        stop=True,
                        skip_group_check=True,
                    )
                    tile.add_dep_helper(
                        tr.ins, mm0.ins, reason="transpose after bank clear"
                    )
                # stage upper blocks needed by later chunks before e_t is freed
                for mc in range(m + 1, NCC):
                    blk = eblk_pool.tile([128, 128], f32, name="eblk", tag="eblk")
                    nc.scalar.copy(blk[:], e_t[:, mc * 128 : (mc + 1) * 128])
                    eblk_sb[(m, mc)] = blk
                m_t = st_pool.tile([128, 1], f32)
                nc.vector.tensor_reduce(m_t[:], e_t[:], axis=AX.X, op=OP.min)
                p_t = prow_pool.tile([128, C], bf16, name="p_t", tag="p_t")
                s_t = st_pool.tile([128, 1], f32)
                nc.scalar.activation(
                    p_t[:], e_t[:], ACT.Exp, bias=m_t[:], scale=-1.0, accum_out=s_t[:]
                )
                r_t = st_pool.tile([128, 1], f32)
                nc.vector.reciprocal(r_t[:], s_t[:])
                t_t = st_pool.tile([128, 1], f32, name="t_t", tag="t_t", bufs=16)
                nc.vector.tensor_scalar_mul(t_t[:], r_t[:], gamma_bc[:])
                t_ts.append(t_t)
                psc_sb.append(p_t)

            # --- x loads; emitted after mm1 so xt loads win early DMA
            # contention; consumers are mm2 (x8) and the epilogue (xb) ---
            xbp = []
            x8p = []
            for j in range(NDP):
                tb = xb_pool.tile([128, 2, L], bf16, name="xbp", tag="xbp")
                nc.sync.dma_start(
                    tb[:],
                    xbd.ap()[b, 256 * j : 256 * (j + 1), :].rearrange(
                        "(k p) l -> p k l", p=128
                    ),
                )
                xbp.append(tb)
                t8 = x8_pool.tile([128, 2, L], fp8, name="x8p", tag="x8p")
                nc.scalar.dma_start(
                    t8[:],
                    x8d.ap()[b, 256 * j : 256 * (j + 1), :].rearrange(
                        "(k p) l -> p k l", p=128
                    ),
                )
                x8p.append(t8)

            # --- software pipeline: run the PT/mm2/epilogue stage for the
            # PREVIOUS batch now, so batch b's softmax DVE ops precede batch
            # b-1's epilogue stt in the in-order DVE queue and the e-bank
            # recycle wait of batch b+1's mm1 lands mid-queue ---
            if pending_stage is not None:
                emit_tail(*pending_stage)
            pending_stage = (b, psc_sb, t_ts, xbp, x8p)

        if pending_stage is not None:
            emit_tail(*pending_stage)

    nc.compile()
    return nc


def _get_nc():
    if "nc" not in _CACHE:
        _CACHE["nc"] = build_nc(BPC)
    return _CACHE["nc"]


def _prep_inputs(x: np.ndarray, gamma: np.ndarray):
    x = np.ascontiguousarray(np.asarray(x, dtype=np.float32))
    gamma = np.asarray(gamma, dtype=np.float32).reshape(1, 1)
    NLT = L // 128
    xb = x.astype(ml_dtypes.bfloat16)
    x8 = x.astype(ml_dtypes.float8_e4m3)
    # xt8[b, p, i*C + c] = x[b, c, 128*i + p]
    xt8 = np.ascontiguousarray(
        x.reshape(B, C, NLT, 128).transpose(0, 3, 2, 1)
    ).astype(ml_dtypes.float8_e4m3).reshape(B, 128, NLT * C)
    in_maps = []
    for c in range(N_CORES):
        sl = slice(c * BPC, (c + 1) * BPC)
        in_maps.append(
            {
                "xb": np.ascontiguousarray(xb[sl]),
                "xt8": np.ascontiguousarray(xt8[sl]),
                "x8": np.ascontiguousarray(x8[sl]),
                "gamma": gamma,
            }
        )
    return in_maps


def kernel(x: np.ndarray, gamma: np.ndarray) -> np.ndarray:
    from concourse.bass_utils import run_bass_kernel_spmd

    nc = _get_nc()
    in_maps = _prep_inputs(x, gamma)
    res = run_bass_kernel_spmd(nc, in_maps, core_ids=list(range(N_CORES)))
    y = np.concatenate([res.results[c]["y"] for c in range(N_CORES)], axis=0)
    return y.astype(np.float32)


def _make_exec_jit(nc, in_specs_names, out_shape, out_dtype=ml_dtypes.bfloat16):
    """One-bass_exec jit over 8 cores, mirroring run_bass_via_pjrt."""
    import jax
    from jax.sharding import Mesh, PartitionSpec
    from jax.experimental.shard_map import shard_map
    from concourse.bass2jax import (
        _bass_exec_p,
        install_neuronx_cc_hook,
        partition_id_tensor,
    )

    install_neuronx_cc_hook()
    out_aval = jax.core.ShapedArray(out_shape, out_dtype)
    out_name = in_specs_names[-1]

    def body(*args):
        outs = _bass_exec_p.bind(
            *args,
            partition_id_tensor(),
            out_avals=(out_aval,),
            in_names=tuple(in_specs_names) + ("partition_id",),
            out_names=(out_name,),
            lowering_input_output_aliases=(),
            sim_require_finite=True,
            sim_require_nnan=True,
            nc=nc,
        )
        return outs[0]

    mesh = Mesh(np.asarray(jax.devices()[:N_CORES]), ("core",))
    spec = PartitionSpec("core")
    jitted = jax.jit(
        shard_map(
            body,
            mesh=mesh,
            in_specs=(spec,) * len(in_specs_names),
            out_specs=spec,
            check_rep=False,
        ),
        keep_unused=True,
    )
    sharding = jax.sharding.NamedSharding(mesh, spec)
    return jitted, sharding


if __name__ == "__main__":
    rng = np.random.default_rng(0)
    x = rng.standard_normal((B, C, L), dtype=np.float32)
    gamma = np.zeros((1,), np.float32)
    y = kernel(x, gamma)
    exp = x.astype(ml_dtypes.bfloat16).astype(np.float32)
    print("gamma=0 matches bf16(x):", np.array_equal(y, exp))
    print("max abs err vs x:", np.abs(y - x).max())
